# revision 1
# baseline (speedup 1.0000x reference)
"""Trainium2 Bass kernel for nn_AttentionBaseline (ragged_sequence).

Data-parallel over batch: 64 batch elements -> 8 cores x 8 elements.
Weights replicated. Each core processes its words shard [8, 2048, 512]:

  mask    = sign(|sum_e words|)            (row validity)
  context = (sum_s words) / n_valid        (mean over valid rows)
  h       = tanh(words @ Wa_top + context @ Wa_bot + b_att)
  scores  = h @ v   (masked softmax over s)
  rep     = attn @ words
  pred    = MLP(rep)

Layout strategy per batch element b (S=2048, E=H=512):
  - WN  [128, 16*512]   words natural   (partition = s%128, free = (s//128, e))
  - WT[ec] [128, 2048]  words transposed (partition = e%128, free = s), via
    64 PE transposes; the PSUM->SBUF copies also accumulate colsum (context).
  - UT = Wa_top^T @ words^T computed as [h-chunk, s] PSUM tiles with fp32r
    matmuls (full PE rate at N=512); tanh+(context-term+bias) fused on ACT.
  - scores via v-stationary matvecs into one PSUM bank (4 row groups),
    reshaped to column layout [128, 16] with K=1 broadcast matmuls.
  - softmax in column layout; cross-partition max/sum via gpsimd
    partition_all_reduce.
  - rep via p-stationary matvecs over WN; MLP on [e, b] columns at the end.
"""

import os
import sys

import numpy as np

for _p in ("/root/.axon_site", "/root/.axon_site/_ro/trn_rl_repo", "/opt/trn_rl_repo"):
    if os.path.isdir(_p) and _p not in sys.path:
        sys.path.append(_p)

import concourse.bass as bass
import concourse.mybir as mybir
import concourse.tile as tile
from concourse import bacc, bass_isa
from concourse.bass_utils import run_bass_kernel_spmd
from concourse.masks import make_identity

F32 = mybir.dt.float32
F32R = mybir.dt.float32r

B_CORE = 8      # batch elements per core
S = 2048        # max set size
E = 512         # embedding dim
H = 512         # hidden dim
T = 128         # target dim
NC_ = 16        # s-chunks of 128
EC = 4          # e-chunks of 128
HC = 4          # h-chunks of 128
NEG_BIG = -1e30


def r(ap):
    """View an fp32 AP as float32r for full-rate PE matmuls."""
    return ap.bitcast(F32R)


def build_kernel(nc, loop_iters=1, level=6):
    words_d = nc.dram_tensor("words", [B_CORE, S, E], F32R, kind="ExternalInput")
    watt_d = nc.dram_tensor("W_att", [2 * E, H], F32R, kind="ExternalInput")
    batt_d = nc.dram_tensor("b_att", [H], F32, kind="ExternalInput")
    v_d = nc.dram_tensor("v", [H, 1], F32R, kind="ExternalInput")
    w1_d = nc.dram_tensor("W1", [E, H], F32, kind="ExternalInput")
    b1_d = nc.dram_tensor("b1", [H], F32, kind="ExternalInput")
    w2_d = nc.dram_tensor("W2", [H, H], F32, kind="ExternalInput")
    b2_d = nc.dram_tensor("b2", [H], F32, kind="ExternalInput")
    w3_d = nc.dram_tensor("W3", [H, T], F32, kind="ExternalInput")
    b3_d = nc.dram_tensor("b3", [T], F32, kind="ExternalInput")
    pred_d = nc.dram_tensor("pred", [B_CORE, T], F32, kind="ExternalOutput")

    from contextlib import ExitStack
    with tile.TileContext(nc) as tc, ExitStack() as ctx:
        const = ctx.enter_context(tc.tile_pool(name="const", bufs=1))
        wn_pool = ctx.enter_context(tc.tile_pool(name="wn", bufs=2))
        wt_pool = ctx.enter_context(tc.tile_pool(name="wt", bufs=1))
        ht_pool = ctx.enter_context(tc.tile_pool(name="ht", bufs=4))
        sm_pool = ctx.enter_context(tc.tile_pool(name="small", bufs=2))
        rep_pool = ctx.enter_context(tc.tile_pool(name="rep", bufs=1))

        pt_pool = ctx.enter_context(tc.tile_pool(name="pst", bufs=2, space="PSUM"))
        pu_pool = ctx.enter_context(tc.tile_pool(name="psu", bufs=3, space="PSUM"))
        prep_pool = ctx.enter_context(tc.tile_pool(name="psrep", bufs=1, space="PSUM"))
        psc_pool = ctx.enter_context(tc.tile_pool(name="pssc", bufs=1, space="PSUM"))
        ptiny_pool = ctx.enter_context(tc.tile_pool(name="pstiny", bufs=1, space="PSUM"))
        dram_pool = ctx.enter_context(tc.tile_pool(name="dscr", bufs=2, space="DRAM"))

        # ---- constants / weights ----
        ident = const.tile([128, 128], F32R, tag="ident")
        nc.gpsimd.memset(ident[:].bitcast(F32), 0.0)
        nc.gpsimd.affine_select(
            out=ident[:], in_=ident[:],
            compare_op=mybir.AluOpType.not_equal,
            fill=1.0, base=0,
            pattern=[[-1, 128]], channel_multiplier=1,
        )
        ones = const.tile([128, 1], F32, tag="ones")
        nc.vector.memset(ones[:], 1.0)

        waT = []  # Wa_top chunks [K=e 128, M=h 512]
        waB = []  # Wa_bot chunks
        w1c, w2c, w3c = [], [], []
        for ec in range(EC):
            t_ = const.tile([128, H], F32R, tag=f"waT{ec}")
            nc.sync.dma_start(t_[:], watt_d[ec * 128:(ec + 1) * 128, :])
            waT.append(t_)
            t_ = const.tile([128, H], F32R, tag=f"waB{ec}")
            nc.sync.dma_start(t_[:], watt_d[E + ec * 128:E + (ec + 1) * 128, :])
            waB.append(t_)
            t_ = const.tile([128, H], F32, tag=f"w1{ec}")
            nc.sync.dma_start(t_[:], w1_d[ec * 128:(ec + 1) * 128, :])
            w1c.append(t_)
            t_ = const.tile([128, H], F32, tag=f"w2{ec}")
            nc.sync.dma_start(t_[:], w2_d[ec * 128:(ec + 1) * 128, :])
            w2c.append(t_)
            t_ = const.tile([128, T], F32, tag=f"w3{ec}")
            nc.sync.dma_start(t_[:], w3_d[ec * 128:(ec + 1) * 128, :])
            w3c.append(t_)

        batt = const.tile([128, HC], F32, tag="batt")
        nc.sync.dma_start(batt[:], batt_d.rearrange("(c p) -> p c", p=128))
        b1t = const.tile([128, HC], F32, tag="b1t")
        nc.sync.dma_start(b1t[:], b1_d.rearrange("(c p) -> p c", p=128))
        b2t = const.tile([128, HC], F32, tag="b2t")
        nc.sync.dma_start(b2t[:], b2_d.rearrange("(c p) -> p c", p=128))
        b3t = const.tile([128, 1], F32, tag="b3t")
        nc.sync.dma_start(b3t[:], b3_d.rearrange("(p one) -> p one", one=1))
        vc = const.tile([128, HC], F32R, tag="vc")
        nc.sync.dma_start(vc[:], v_d.rearrange("(c p) one -> p (c one)", p=128))

        rep_all = rep_pool.tile([1, B_CORE * E], F32, tag="rep_all")

        def body():
            prev = run_batches()
            if level >= 6:
                if prev is not None:
                    emit_rep(*prev)
                run_mlp()

        def emit_rep(b, wn, p_sb, recip_z):
            # rep = attn @ words (deferred one batch for pipelining)
            prep = prep_pool.tile([1, E], F32, tag="prep", name="prep")
            for sc in range(NC_):
                nc.tensor.matmul(
                    prep[:],
                    p_sb[:, sc:sc + 1],
                    wn[:, sc * E:(sc + 1) * E],
                    start=(sc == 0), stop=(sc == NC_ - 1),
                )
            for ec in range(EC):
                nc.vector.tensor_scalar(
                    out=rep_all[0:1, ec * 1024 + b * 128: ec * 1024 + (b + 1) * 128],
                    in0=prep[0:1, ec * 128:(ec + 1) * 128],
                    scalar1=recip_z[0:1, 0:1],
                    scalar2=None,
                    op0=mybir.AluOpType.mult,
                )

        def run_batches():
          prev_rep = None
          for b in range(B_CORE):
            # ---- load words[b] (one 4 MB DMA) ----
            wn = wn_pool.tile([128, NC_ * E], F32R, tag="wn")
            # two half-loads: transposes of s-chunks 0-7 start at half-load
            nc.sync.dma_start(
                wn[:, :8 * E].rearrange("p (c e) -> p c e", e=E),
                words_d[b, :8 * 128].rearrange("(c p) e -> p c e", p=128),
            )
            nc.sync.dma_start(
                wn[:, 8 * E:].rearrange("p (c e) -> p c e", e=E),
                words_d[b, 8 * 128:].rearrange("(c p) e -> p c e", p=128),
            )

            if level >= 3:
                # ---- row sums (mask) in column layout [128, 16] ----
                # split DVE (chunks 0-9) / ACT (chunks 10-15) to balance the
                # transpose-window engine load
                rowsum = sm_pool.tile([128, NC_], F32, tag="rowsum")
                for lo, hi in ((0, 4), (4, 8), (8, 10)):
                    nc.vector.tensor_reduce(
                        out=rowsum[:, lo:hi],
                        in_=wn[:, lo * E:hi * E].bitcast(F32).rearrange(
                            "p (c e) -> p c e", e=E
                        ),
                        axis=mybir.AxisListType.X,
                        op=mybir.AluOpType.add,
                    )
                rs_scratch = sm_pool.tile([128, E], F32, tag="rs_scratch")
                for k in range(10, NC_):
                    nc.scalar.activation(
                        out=rs_scratch[:],
                        in_=wn[:, k * E:(k + 1) * E].bitcast(F32),
                        func=mybir.ActivationFunctionType.Identity,
                        accum_out=rowsum[:, k:k + 1],
                    )
                mask01 = sm_pool.tile([128, NC_], F32, tag="mask01")
                nc.vector.tensor_scalar(
                    out=mask01[:],
                    in0=rowsum[:],
                    scalar1=0.0,
                    scalar2=None,
                    op0=mybir.AluOpType.not_equal,
                )
                lpart = sm_pool.tile([128, 1], F32, tag="lpart")
                nc.vector.tensor_reduce(
                    out=lpart[:], in_=mask01[:],
                    axis=mybir.AxisListType.X, op=mybir.AluOpType.add,
                )
                lall = sm_pool.tile([128, 1], F32, tag="lall")
                nc.gpsimd.partition_all_reduce(
                    lall[:], lpart[:], channels=128, reduce_op=bass_isa.ReduceOp.add
                )
                recip_l = sm_pool.tile([128, 1], F32, tag="recip_l")
                nc.vector.reciprocal(recip_l[:], lall[:])
            if level < 2:
                continue
            # ---- transpose words into WT[ec] [e 128, s 2048] ----
            wt = [
                wt_pool.tile([128, S], F32R, tag=f"wt{ec}", name=f"wt{ec}")
                for ec in range(EC)
            ]
            csum_parts = sm_pool.tile([128, 16], F32, tag="csum_parts")
            for scg in range(4):
                for ec in range(EC):
                    pt = pt_pool.tile([128, 512], F32, tag="pt")
                    for q in range(4):
                        sc = scg * 4 + q
                        nc.tensor.transpose(
                            r(pt[:, q * 128:(q + 1) * 128]),
                            wn[:, sc * E + ec * 128: sc * E + (ec + 1) * 128],
                            ident[:],
                        )
                    # copy to SBUF; accumulate colsum partial over free dim
                    if scg % 2 == 0:
                        nc.vector.tensor_scalar(
                            out=wt[ec][:, scg * 512:(scg + 1) * 512],
                            in0=pt[:],
                            scalar1=0.0,
                            scalar2=None,
                            op0=mybir.AluOpType.add,
                            op1=mybir.AluOpType.add,
                            accum_out=csum_parts[:, ec * 4 + scg: ec * 4 + scg + 1],
                        )
                    else:
                        nc.scalar.activation(
                            out=wt[ec][:, scg * 512:(scg + 1) * 512],
                            in_=pt[:],
                            func=mybir.ActivationFunctionType.Identity,
                            accum_out=csum_parts[:, ec * 4 + scg: ec * 4 + scg + 1],
                        )


            # ---- context & its attention-layer contribution ----
            csum = sm_pool.tile([128, EC], F32, tag="csum")
            nc.vector.tensor_reduce(
                out=csum[:],
                in_=csum_parts[:].rearrange("p (ec scg) -> p ec scg", scg=4),
                axis=mybir.AxisListType.X, op=mybir.AluOpType.add,
            )
            ctx = sm_pool.tile([128, EC], F32, tag="ctx")
            nc.vector.tensor_scalar(
                out=ctx[:], in0=csum[:], scalar1=recip_l[:, 0:1], scalar2=None,
                op0=mybir.AluOpType.mult,
            )
            pcv = ptiny_pool.tile([128, HC], F32, tag="tiny", padded_shape=[128, 32])
            for hc in range(HC):
                for ec in range(EC):
                    nc.tensor.matmul(
                        pcv[:, hc:hc + 1],
                        waB[ec][:, hc * 128:(hc + 1) * 128].bitcast(F32),
                        ctx[:, ec:ec + 1],
                        start=(ec == 0), stop=(ec == EC - 1),
                    )
            cvb = sm_pool.tile([128, HC], F32, tag="cvb")
            nc.vector.tensor_add(cvb[:], pcv[:], batt[:])

            if level < 4:
                continue
            # ---- main matmul UT = Wa_top^T @ words^T, tanh, scores ----
            sc_sb = sm_pool.tile([1, S], F32, tag="sc_sb")
            for sblk in range(4):
                psc = psc_pool.tile([1, 512], F32, tag="psc")
                hts = []
                for hc in range(HC):
                    pu = pu_pool.tile([128, 512], F32, tag="pu")
                    for ec in range(EC):
                        nc.tensor.matmul(
                            pu[:],
                            waT[ec][:, hc * 128:(hc + 1) * 128],
                            wt[ec][:, sblk * 512:(sblk + 1) * 512],
                            start=(ec == 0), stop=(ec == EC - 1),
                        )
                    ht = ht_pool.tile([128, 512], F32R, tag="ht",
                                      name=f"ht{hc}")
                    nc.scalar.activation(
                        out=ht[:], in_=pu[:],
                        func=mybir.ActivationFunctionType.Tanh,
                        bias=cvb[:, hc:hc + 1],
                    )
                    hts.append(ht)
                if level >= 5:
                    for hc in range(HC):
                        nc.tensor.matmul(
                            psc[:],
                            vc[:, hc:hc + 1],
                            hts[hc][:],
                            start=(hc == 0), stop=(hc == HC - 1),
                        )
                    nc.vector.tensor_copy(
                        sc_sb[0:1, sblk * 512:(sblk + 1) * 512], psc[:]
                    )

            if level < 6:
                continue
            # ---- scores -> column layout [128, 16] ----
            pst = sm_pool.tile([128, NC_], F32, tag="pst")
            sc_dr = dram_pool.tile([1, S], F32, tag="sc_dr")
            nc.sync.dma_start(sc_dr[:], sc_sb[:])
            nc.sync.dma_start(
                pst[:], sc_dr[0].rearrange("(k p) -> p k", p=128)
            )
            mask_i = sm_pool.tile([128, NC_], mybir.dt.int32, tag="mask_i")
            nc.vector.tensor_scalar(
                out=mask_i[:], in0=rowsum[:], scalar1=0.0, scalar2=None,
                op0=mybir.AluOpType.not_equal,
            )
            smask = sm_pool.tile([128, NC_], F32, tag="smask")
            nc.vector.memset(smask[:], NEG_BIG)
            nc.vector.copy_predicated(smask[:], mask_i[:], pst[:])

            # ---- masked softmax (column layout) ----
            mpart = sm_pool.tile([128, 1], F32, tag="mpart")
            nc.vector.tensor_reduce(
                out=mpart[:], in_=smask[:],
                axis=mybir.AxisListType.X, op=mybir.AluOpType.max,
            )
            mall = sm_pool.tile([128, 1], F32, tag="mall")
            nc.gpsimd.partition_all_reduce(
                mall[:], mpart[:], channels=128, reduce_op=bass_isa.ReduceOp.max
            )
            negm = sm_pool.tile([128, 1], F32, tag="negm")
            nc.vector.tensor_scalar(
                out=negm[:], in0=mall[:], scalar1=-1.0, scalar2=None,
                op0=mybir.AluOpType.mult,
            )
            p_sb = sm_pool.tile([128, NC_], F32R, tag="p_sb", bufs=3)
            zpart = sm_pool.tile([128, 1], F32, tag="zpart")
            nc.scalar.activation(
                out=p_sb[:], in_=smask[:],
                func=mybir.ActivationFunctionType.Exp,
                bias=negm[:, 0:1],
                accum_out=zpart[:],
            )
            zall = sm_pool.tile([128, 1], F32, tag="zall")
            nc.gpsimd.partition_all_reduce(
                zall[:], zpart[:], channels=128, reduce_op=bass_isa.ReduceOp.add
            )
            recip_z = sm_pool.tile([128, 1], F32, tag="recip_z", bufs=3)
            nc.vector.reciprocal(recip_z[:], zall[:])

            if prev_rep is not None:
                emit_rep(*prev_rep)
            prev_rep = (b, wn, p_sb, recip_z)
          return prev_rep

        def run_mlp():
            # ---- MLP over all 8 reps ----
            # repT [e 128, (ec, b)] via K=1 broadcast matmuls
            rT = sm_pool.tile([128, EC * B_CORE], F32, tag="rT")
            rep_dr = dram_pool.tile([1, B_CORE * E], F32, tag="rep_dr")
            nc.sync.dma_start(rep_dr[:], rep_all[:])
            nc.sync.dma_start(
                rT[:].rearrange("p (ec b) -> p ec b", b=B_CORE),
                rep_dr[0].rearrange("(ec b p) -> p ec b", p=128, b=B_CORE),
            )

            h1 = sm_pool.tile([128, HC * B_CORE], F32, tag="h1")
            for hc in range(HC):
                pm = ptiny_pool.tile([128, B_CORE], F32, tag="tiny", padded_shape=[128, 32])
                for ec in range(EC):
                    nc.tensor.matmul(
                        pm[:],
                        w1c[ec][:, hc * 128:(hc + 1) * 128],
                        rT[:, ec * B_CORE:(ec + 1) * B_CORE],
                        start=(ec == 0), stop=(ec == EC - 1),
                    )
                nc.scalar.activation(
                    out=h1[:, hc * B_CORE:(hc + 1) * B_CORE], in_=pm[:],
                    func=mybir.ActivationFunctionType.Relu,
                    bias=b1t[:, hc:hc + 1],
                )
            h2 = sm_pool.tile([128, HC * B_CORE], F32, tag="h2")
            for hc in range(HC):
                pm = ptiny_pool.tile([128, B_CORE], F32, tag="tiny", padded_shape=[128, 32])
                for ec in range(EC):
                    nc.tensor.matmul(
                        pm[:],
                        w2c[ec][:, hc * 128:(hc + 1) * 128],
                        h1[:, ec * B_CORE:(ec + 1) * B_CORE],
                        start=(ec == 0), stop=(ec == EC - 1),
                    )
                nc.scalar.activation(
                    out=h2[:, hc * B_CORE:(hc + 1) * B_CORE], in_=pm[:],
                    func=mybir.ActivationFunctionType.Relu,
                    bias=b2t[:, hc:hc + 1],
                )
            po = ptiny_pool.tile([128, B_CORE], F32, tag="tiny", padded_shape=[128, 32])
            for ec in range(EC):
                nc.tensor.matmul(
                    po[:],
                    w3c[ec][:],
                    h2[:, ec * B_CORE:(ec + 1) * B_CORE],
                    start=(ec == 0), stop=(ec == EC - 1),
                )
            out_sb = sm_pool.tile([128, B_CORE], F32, tag="out_sb")
            nc.scalar.activation(
                out=out_sb[:], in_=po[:],
                func=mybir.ActivationFunctionType.Identity,
                bias=b3t[:, 0:1],
            )
            nc.sync.dma_start(pred_d.rearrange("b t -> t b"), out_sb[:])

        if loop_iters > 1:
            with tc.For_i(0, loop_iters, 1):
                body()
        else:
            body()

    return nc


_NC = None


def get_nc(loop_iters=1):
    global _NC
    if _NC is None:
        nc = bacc.Bacc("TRN2", target_bir_lowering=False, debug=False,
                       num_devices=8)
        build_kernel(nc, loop_iters=loop_iters)
        nc.compile()
        _NC = nc
    return _NC


def kernel(**inputs):
    words = np.ascontiguousarray(np.asarray(inputs["words"], dtype=np.float32))
    assert words.shape == (64, 2048, 512), words.shape
    weights = {
        k: np.ascontiguousarray(np.asarray(inputs[k], dtype=np.float32))
        for k in ("W_att", "b_att", "v", "W1", "b1", "W2", "b2", "W3", "b3")
    }
    nc = get_nc()
    in_maps = []
    for c in range(8):
        m = {"words": words[c * B_CORE:(c + 1) * B_CORE]}
        m.update(weights)
        in_maps.append(m)
    res = run_bass_kernel_spmd(nc, in_maps, list(range(8)))
    out = np.concatenate([res.results[c]["pred"] for c in range(8)], axis=0)
    return out.astype(np.float32)


if __name__ == "__main__":
    # smoke test with random data
    rng = np.random.default_rng(0)
    ins = {
        "words": rng.standard_normal((64, 2048, 512), dtype=np.float32),
        "W_att": rng.standard_normal((1024, 512), dtype=np.float32) * 0.03,
        "b_att": rng.standard_normal((512,), dtype=np.float32) * 0.03,
        "v": rng.standard_normal((512, 1), dtype=np.float32),
        "W1": rng.standard_normal((512, 512), dtype=np.float32) * 0.04,
        "b1": rng.standard_normal((512,), dtype=np.float32) * 0.04,
        "W2": rng.standard_normal((512, 512), dtype=np.float32) * 0.04,
        "b2": rng.standard_normal((512,), dtype=np.float32) * 0.04,
        "W3": rng.standard_normal((512, 128), dtype=np.float32) * 0.04,
        "b3": rng.standard_normal((128,), dtype=np.float32) * 0.04,
    }
    out = kernel(**ins)
    print("out", out.shape, out.dtype, np.abs(out).mean())

